# revision 16
# baseline (speedup 1.0000x reference)
"""ContraNorm kernel for 8 Trainium2 NeuronCores — fp8 DoubleRow pipeline.

Math (reference):
    norm_x = x / max(||x||_row, eps)
    sim    = (norm_x @ norm_x.T) / tau          # [N, N], tau = 1
    sim[edge_index[0], edge_index[1]] = -inf
    attn   = softmax(sim, axis=1)
    out    = 1.1 * x - 0.1 * (attn @ x)

Sharding: row-parallel.  Core k owns output rows [k*1024, (k+1)*1024).
Each core receives inputs row-rolled so its own rows sit at c-positions
0:1024 — the program is identical on every core (pure SPMD).

Since sim is a cosine similarity in [-1, 1], softmax needs no running
max: exp(sim) is in [e^-1, e].  The -inf edge mask becomes an exact
multiply of exp(sim) by {0, 1}, applied as an integer byte multiply on
the fp8 bit patterns.  The row-sum comes from a ones-column appended to
the V-matmul rhs.

fp8 (e4m3) everywhere on the matmul paths, with DoubleRow perf mode:
  sim:  psum[c,m] = sum_{kt,dp} xt[dp,kt,c] * xt[dp,kt,m]   1 MM / c-chunk
  V:    pv[m,:]  += sum_{kt,cp} et2[cp,kt,m] * xa[cp,kt,:]  4 MM / c-pair
norm_x is pre-scaled by 16 on the host (entries ~N(0,1) in fp8); the
exp activation rescales by 1/256.

The edge mask ships bit-packed (1 MiB/core) and is expanded on-chip
with one fused (x shift) & const op per bit on u32 lanes (DVE-only:
the Pool engine has no integer/bitwise ops).  Pairs are split 2:1
between DVE and GpSimd so the exp activation stays the pacer:
  DVE pairs:    exp bias 0, et = e^sim in [0.37, 2.72]; mask bytes
                {0x00, 0x01} applied as a u8 integer multiply (x{0,1}
                on the raw fp8 bit patterns - integer ops are legal on
                DVE only).
  GpSimd pairs: exp bias ln 64, et = 64 e^sim in [23.5, 174]; mask
                bytes {0x00, 0x08} = fp8 {0, 2^-6} applied as an fp8
                float multiply - an exact exponent shift back to
                e^sim.
Both land et at e^sim, so the V accumulation is scale-uniform.  The
host pre-permutes bit chunks so each engine's mask region is
contiguous (DVE chunks t%6<4 first, GpSimd chunks t%6>=4 last).

Per-core inputs (6.07 MiB vs 21 MiB for the dense-bf16-mask variant):
  xt   [128, 2, 8192] fp8   16*norm_x rolled, transposed
  xa   [128, 32, 2, 257] fp8  x rolled (V rhs layout) + ones column
  bits [128, 64, 2, 16] u32  keep-mask bits, b*64+j column mapping
  xo   [1024, 256] f32      own rows for the 1.1*x epilogue term
"""

import numpy as np
import ml_dtypes

N = 8192          # rows of x
D = 256           # features
P = 128           # SBUF partitions
NT = N // P       # 64 c-chunks
R = N // 8        # 1024 rows per core
HALF = 512        # m columns per pass
NPAIR = NT // 2   # 32 c-chunk pairs
SCALE = 0.1
NCORES = 8

# pair g -> GpSimd iff g % 3 == 2; chunk t belongs to pair t//2
GP_PAIRS = [g for g in range(NPAIR) if g % 3 == 2]
NTG = 2 * len(GP_PAIRS)    # chunks masked on GpSimd (20)
NTD = NT - NTG             # chunks masked on DVE (44)
CHUNK_ORDER = [t for t in range(NT) if (t // 2) % 3 != 2] + [
    t for t in range(NT) if (t // 2) % 3 == 2
]

_prog_cache = {}


def _build_program():
    import concourse.bacc as bacc
    import concourse.tile as tile
    from concourse import mybir
    from contextlib import ExitStack

    f32 = mybir.dt.float32
    fp8 = mybir.dt.float8e4
    u32 = mybir.dt.uint32
    u8 = mybir.dt.uint8
    DR = mybir.MatmulPerfMode.DoubleRow
    Exp = mybir.ActivationFunctionType.Exp
    SHR = mybir.AluOpType.logical_shift_right
    SHL = mybir.AluOpType.logical_shift_left
    AND = mybir.AluOpType.bitwise_and
    MUL = mybir.AluOpType.mult
    ADD = mybir.AluOpType.add

    nc = bacc.Bacc("TRN2", target_bir_lowering=False, debug=False)

    xt_h = nc.dram_tensor("xt", [P, 2, N], fp8, kind="ExternalInput")
    xa_h = nc.dram_tensor("xa", [P, NPAIR, 2, D + 1], fp8, kind="ExternalInput")
    bits_h = nc.dram_tensor("bits", [P, NT, 2, 16], u32, kind="ExternalInput")
    xo_h = nc.dram_tensor("xo", [R, D], f32, kind="ExternalInput")
    out_h = nc.dram_tensor("out", [R, D], f32, kind="ExternalOutput")

    xo_d = xo_h.ap().rearrange("(j p) d -> p j d", p=P)    # [128, 8, 256]
    out_d = out_h.ap()

    with ExitStack() as ctx:
        tc = ctx.enter_context(tile.TileContext(nc))

        consts = ctx.enter_context(tc.tile_pool(name="consts", bufs=1))
        maskp = ctx.enter_context(tc.tile_pool(name="maskp", bufs=2))
        work = ctx.enter_context(tc.tile_pool(name="work", bufs=4))
        ps_s = ctx.enter_context(tc.tile_pool(name="ps_s", bufs=2, space="PSUM"))
        ps_v = ctx.enter_context(tc.tile_pool(name="ps_v", bufs=1, space="PSUM"))

        xt = consts.tile([P, 2, N], fp8)
        xa = consts.tile([P, NPAIR, 2, D + 1], fp8)
        bits = consts.tile([P, NT, 2, 16], u32)
        xo = consts.tile([P, R // P, D], f32)
        ebias = consts.tile([P, 1], f32)
        nc.gpsimd.memset(ebias, float(np.log(64.0)))

        # bits first (mask expansion is the first compute), then the
        # matmul operands in chunks so compute starts early.
        nc.sync.dma_start(out=bits, in_=bits_h.ap())
        nc.sync.dma_start(out=xt[:, :, 0:R], in_=xt_h.ap()[:, :, 0:R])
        nc.sync.dma_start(out=xt[:, :, R:N], in_=xt_h.ap()[:, :, R:N])
        NXA = 4
        for q in range(NXA):
            sl = slice(q * (NPAIR // NXA), (q + 1) * (NPAIR // NXA))
            nc.sync.dma_start(out=xa[:, sl], in_=xa_h.ap()[:, sl])
        nc.sync.dma_start(out=xo, in_=xo_d)

        for h in range(2):
            m0 = h * HALF
            # ---- expand this half's mask bits to {0,1} bytes ----
            mexp = maskp.tile([P, NT, HALF], fp8, tag="mexp")
            mexp32 = mexp.bitcast(u32)           # [P, NT, 128]
            for b in range(8):
                # DVE region (first NTD chunks): bit b -> byte 0x01
                nc.vector.tensor_scalar(
                    out=mexp32[:, 0:NTD, b * 16 : (b + 1) * 16],
                    in0=bits[:, 0:NTD, h, :],
                    scalar1=b,
                    scalar2=0x01010101,
                    op0=SHR,
                    op1=AND,
                )
                # GpSimd region (last NTG chunks): bit b -> byte 0x08
                nc.vector.tensor_scalar(
                    out=mexp32[:, NTD:NT, b * 16 : (b + 1) * 16],
                    in0=bits[:, NTD:NT, h, :],
                    scalar1=(3 - b) if b < 3 else (b - 3),
                    scalar2=0x08080808,
                    op0=SHL if b < 3 else SHR,
                    op1=AND,
                )

            pv = [
                ps_v.tile([P, D + 1], f32, tag=f"pv{mi}", name=f"pv{mi}")
                for mi in range(4)
            ]
            for g in range(NPAIR):
                pss = ps_s.tile([P, 2, HALF], f32, tag="pss")
                for kt in range(2):
                    t = 2 * g + kt
                    nc.tensor.matmul(
                        pss[:, kt, :],
                        xt[:, :, t * P : (t + 1) * P],
                        xt[:, :, m0 : m0 + HALF],
                        start=True,
                        stop=True,
                        perf_mode=DR,
                    )
                is_gp = g % 3 == 2
                et2 = work.tile([P, 2, HALF], fp8, tag="et2", bufs=6)
                nc.scalar.activation(
                    et2.rearrange("p a b -> p (a b)"),
                    pss.rearrange("p a b -> p (a b)"),
                    Exp,
                    scale=1.0 / 256.0,
                    bias=ebias if is_gp else 0.0,
                )
                # mask apply, out-of-place
                et2m = work.tile([P, 2, HALF], fp8, tag="et2m", bufs=6)
                if is_gp:
                    # fp8 float multiply by {0, 2^-6}
                    j = NTD + 2 * (g // 3)
                    nc.gpsimd.tensor_tensor(
                        out=et2m.rearrange("p a b -> p (a b)"),
                        in0=et2.rearrange("p a b -> p (a b)"),
                        in1=mexp[:, j : j + 2, :].rearrange(
                            "p a b -> p (a b)"
                        ),
                        op=MUL,
                    )
                else:
                    # u8 integer multiply by {0, 1}
                    j = 2 * (g - (g + 1) // 3)
                    nc.vector.tensor_tensor(
                        out=et2m.rearrange("p a b -> p (a b)").bitcast(u8),
                        in0=et2.rearrange("p a b -> p (a b)").bitcast(u8),
                        in1=mexp[:, j : j + 2, :].rearrange(
                            "p a b -> p (a b)"
                        ).bitcast(u8),
                        op=MUL,
                    )
                for mi in range(4):
                    nc.tensor.matmul(
                        pv[mi],
                        et2m[:, :, mi * P : (mi + 1) * P],
                        xa[:, g],
                        start=(g == 0),
                        stop=(g == NPAIR - 1),
                        perf_mode=DR,
                    )
            # ---- epilogue: out = 1.1*x - 0.1 * pv/S ----
            for mi in range(4):
                jj = h * 4 + mi
                sinv = work.tile([P, 1], f32, tag="sinv")
                nc.vector.reciprocal(sinv, pv[mi][:, D : D + 1])
                res = work.tile([P, D], f32, tag="res")
                nc.vector.tensor_scalar(
                    out=res,
                    in0=pv[mi][:, 0:D],
                    scalar1=sinv,
                    scalar2=-SCALE,
                    op0=MUL,
                    op1=MUL,
                )
                nc.vector.scalar_tensor_tensor(
                    out=res,
                    in0=xo[:, jj],
                    scalar=1.0 + SCALE,
                    in1=res,
                    op0=MUL,
                    op1=ADD,
                )
                nc.sync.dma_start(
                    out=out_d[jj * P : (jj + 1) * P, :], in_=res
                )

    nc.compile()
    return nc


def get_program():
    if "prog" not in _prog_cache:
        _prog_cache["prog"] = _build_program()
    return _prog_cache["prog"]


def make_in_maps(x, edge_index):
    fp8 = ml_dtypes.float8_e4m3
    x = np.asarray(x, dtype=np.float32)
    ei = np.asarray(edge_index)
    r = ei[0].astype(np.int64)
    c = ei[1].astype(np.int64)

    norm = np.sqrt((x * x).sum(axis=1, keepdims=True))
    nx16 = np.asarray((x / np.maximum(norm, 1e-12)) * 16.0, dtype=fp8)
    x8 = np.asarray(x, dtype=fp8)

    in_maps = []
    for k in range(NCORES):
        lo = k * R
        nxr = np.roll(nx16, -lo, axis=0)          # [N, D] fp8
        xar = np.roll(x8, -lo, axis=0)            # [N, D] fp8

        # xt[p, kt, c] = nxr[c, kt*128 + p]
        xt = np.ascontiguousarray(
            nxr.T.reshape(2, P, N).transpose(1, 0, 2)
        )
        # xa[p, g, kt, j] = xar[(2g+kt)*128 + p, j], ones at j=256
        xa = np.empty((P, NPAIR, 2, D + 1), dtype=fp8)
        xa[:, :, :, 0:D] = xar.reshape(NPAIR, 2, P, D).transpose(2, 0, 1, 3)
        xa[:, :, :, D] = fp8(1.0)

        # keep-mask, rolled: mask[c_rolled, m_local] = 0 on edges
        sel = (r >= lo) & (r < lo + R)
        m_local = (r[sel] - lo).astype(np.int64)
        c_rolled = (c[sel] - lo) % N
        mask = np.ones((N, R), dtype=np.uint8)
        mask[c_rolled, m_local] = 0
        # column mapping m = h*512 + b*64 + j  ->  byte[c, h, j] bit b
        mm = mask.reshape(N, 2, 8, 64)
        packed = np.packbits(mm, axis=2, bitorder="little")  # [N, 2, 1, 64]
        packed = packed.reshape(N, 2, 64)
        # bits[p, t, h, w] = u32 view of packed[t*128+p, h, 4w:4w+4]
        bits = (
            packed.reshape(NT, P, 2, 64)[CHUNK_ORDER]
            .transpose(1, 0, 2, 3)
            .copy()
            .view("<u4")
        )
        xo = np.ascontiguousarray(x[lo : lo + R])
        in_maps.append({"xt": xt, "xa": xa, "bits": bits, "xo": xo})
    return in_maps


def run(x, edge_index, trace=False):
    from concourse.bass_utils import run_bass_kernel_spmd

    nc = get_program()
    in_maps = make_in_maps(x, edge_index)
    br = run_bass_kernel_spmd(nc, in_maps, list(range(NCORES)), trace=trace)
    out = np.concatenate(
        [br.results[k]["out"] for k in range(NCORES)], axis=0
    ).astype(np.float32)
    return out, br


def kernel(x, edge_index):
    out, _ = run(x, edge_index, trace=False)
    return out


# revision 21
# speedup vs baseline: 1.1554x; 1.1554x over previous
"""ContraNorm kernel for 8 Trainium2 NeuronCores — fp8 DoubleRow pipeline.

Math (reference):
    norm_x = x / max(||x||_row, eps)
    sim    = (norm_x @ norm_x.T) / tau          # [N, N], tau = 1
    sim[edge_index[0], edge_index[1]] = -inf
    attn   = softmax(sim, axis=1)
    out    = 1.1 * x - 0.1 * (attn @ x)

Sharding: row-parallel.  Core k owns output rows [k*1024, (k+1)*1024).
Each core receives inputs row-rolled so its own rows sit at c-positions
0:1024 — the program is identical on every core (pure SPMD).

Since sim is a cosine similarity in [-1, 1], softmax needs no running
max: exp(sim) is in [e^-1, e].  The -inf edge mask becomes an exact
multiply of exp(sim) by {0, 1}, applied as an integer byte multiply on
the fp8 bit patterns.  The row-sum comes from a ones-column appended to
the V-matmul rhs.

fp8 (e4m3) everywhere on the matmul paths, with DoubleRow perf mode:
  sim:  psum[c,m] = sum_{kt,dp} xt[dp,kt,c] * xt[dp,kt,m]   1 MM / c-chunk
  V:    pv[m,:]  += sum_{kt,cp} et2[cp,kt,m] * xa[cp,kt,:]  4 MM / c-pair
norm_x is pre-scaled by 16 on the host (entries ~N(0,1) in fp8); the
exp activation rescales by 1/256.

The edge mask ships bit-packed (1 MiB/core) and is expanded on-chip to
{0x00, 0x01} bytes with one fused (x >> b) & 0x01010101 op per bit on
u32 lanes, then applied as a u8 integer multiply on the raw fp8 bit
patterns of exp(sim) (x1 keeps the byte, x0 zeroes it).  All mask work
runs on DVE: GpSimd shares its SBUF port with DVE, so offloading
elementwise work there just steals DVE bandwidth.  The multiply is
batched over GRP pairs per instruction to amortize DVE op overhead.

Per-core inputs (6.07 MiB vs 21 MiB for the dense-bf16-mask variant):
  xt   [128, 2, 8192] fp8   16*norm_x rolled, transposed
  xa   [128, 32, 2, 257] fp8  x rolled (V rhs layout) + ones column
  bits [128, 64, 2, 16] u32  keep-mask bits, b*64+j column mapping
  xo   [1024, 256] f32      own rows for the 1.1*x epilogue term
"""

import numpy as np
import ml_dtypes

N = 8192          # rows of x
D = 256           # features
P = 128           # SBUF partitions
NT = N // P       # 64 c-chunks
R = N // 8        # 1024 rows per core
HALF = 512        # m columns per pass
NPAIR = NT // 2   # 32 c-chunk pairs
SCALE = 0.1
NCORES = 8

GRP = 2           # pairs whose mask multiply is batched into one DVE op

_prog_cache = {}


def _build_program():
    import concourse.bacc as bacc
    import concourse.tile as tile
    from concourse import mybir
    from contextlib import ExitStack

    f32 = mybir.dt.float32
    fp8 = mybir.dt.float8e4
    u32 = mybir.dt.uint32
    u8 = mybir.dt.uint8
    DR = mybir.MatmulPerfMode.DoubleRow
    Exp = mybir.ActivationFunctionType.Exp
    SHR = mybir.AluOpType.logical_shift_right
    SHL = mybir.AluOpType.logical_shift_left
    AND = mybir.AluOpType.bitwise_and
    MUL = mybir.AluOpType.mult
    ADD = mybir.AluOpType.add

    nc = bacc.Bacc("TRN2", target_bir_lowering=False, debug=False)

    xt_h = nc.dram_tensor("xt", [P, 2, N], fp8, kind="ExternalInput")
    xa_h = nc.dram_tensor("xa", [P, NPAIR, 2, D + 1], fp8, kind="ExternalInput")
    bits_h = nc.dram_tensor("bits", [P, NT, 2, 16], u32, kind="ExternalInput")
    xo_h = nc.dram_tensor("xo", [R, D], f32, kind="ExternalInput")
    out_h = nc.dram_tensor("out", [R, D], f32, kind="ExternalOutput")

    xo_d = xo_h.ap().rearrange("(j p) d -> p j d", p=P)    # [128, 8, 256]
    out_d = out_h.ap()

    with ExitStack() as ctx:
        tc = ctx.enter_context(tile.TileContext(nc))

        consts = ctx.enter_context(tc.tile_pool(name="consts", bufs=1))
        maskp = ctx.enter_context(tc.tile_pool(name="maskp", bufs=2))
        work = ctx.enter_context(tc.tile_pool(name="work", bufs=4))
        ps_s = ctx.enter_context(tc.tile_pool(name="ps_s", bufs=2, space="PSUM"))
        ps_v = ctx.enter_context(tc.tile_pool(name="ps_v", bufs=1, space="PSUM"))

        xt = consts.tile([P, 2, N], fp8)
        xa = consts.tile([P, NPAIR, 2, D + 1], fp8)
        bits = consts.tile([P, NT, 2, 16], u32)
        xo = consts.tile([P, R // P, D], f32)

        # bits first (mask expansion is the first compute), then the
        # matmul operands in chunks so compute starts early.
        nc.sync.dma_start(out=bits, in_=bits_h.ap())
        nc.sync.dma_start(out=xt[:, :, 0:R], in_=xt_h.ap()[:, :, 0:R])
        nc.sync.dma_start(out=xt[:, :, R:N], in_=xt_h.ap()[:, :, R:N])
        NXA = 4
        for q in range(NXA):
            sl = slice(q * (NPAIR // NXA), (q + 1) * (NPAIR // NXA))
            nc.sync.dma_start(out=xa[:, sl], in_=xa_h.ap()[:, sl])
        nc.sync.dma_start(out=xo, in_=xo_d)

        for h in range(2):
            m0 = h * HALF
            # ---- expand this half's mask bits to {0,1} bytes ----
            mexp = maskp.tile([P, NT, HALF], fp8, tag="mexp")
            mexp32 = mexp.bitcast(u32)           # [P, NT, 128]
            for b in range(8):
                # bit b -> byte 0x01 ({0,1} u8 mask)
                nc.vector.tensor_scalar(
                    out=mexp32[:, :, b * 16 : (b + 1) * 16],
                    in0=bits[:, :, h, :],
                    scalar1=b,
                    scalar2=0x01010101,
                    op0=SHR,
                    op1=AND,
                )

            pv = [
                ps_v.tile([P, D + 1], f32, tag=f"pv{mi}", name=f"pv{mi}")
                for mi in range(4)
            ]
            for gg in range(NPAIR // GRP):
                # et4 holds GRP pairs of exp tiles; masked in one DVE op
                et4 = work.tile([P, GRP, 2, HALF], fp8, tag="et4", bufs=4)
                et4m = work.tile([P, GRP, 2, HALF], fp8, tag="et4m", bufs=4)
                for gi in range(GRP):
                    g = gg * GRP + gi
                    pss = ps_s.tile([P, 2, HALF], f32, tag="pss")
                    for kt in range(2):
                        t = 2 * g + kt
                        nc.tensor.matmul(
                            pss[:, kt, :],
                            xt[:, :, t * P : (t + 1) * P],
                            xt[:, :, m0 : m0 + HALF],
                            start=True,
                            stop=True,
                            perf_mode=DR,
                        )
                    nc.scalar.activation(
                        et4[:, gi].rearrange("p a b -> p (a b)"),
                        pss.rearrange("p a b -> p (a b)"),
                        Exp,
                        scale=1.0 / 256.0,
                    )
                # mask apply: one u8 integer multiply by {0,1} per group
                t0 = gg * GRP * 2
                nc.vector.tensor_tensor(
                    out=et4m.rearrange("p a k b -> p (a k b)").bitcast(u8),
                    in0=et4.rearrange("p a k b -> p (a k b)").bitcast(u8),
                    in1=mexp[:, t0 : t0 + 2 * GRP, :].rearrange(
                        "p a b -> p (a b)"
                    ).bitcast(u8),
                    op=MUL,
                )
                for gi in range(GRP):
                    g = gg * GRP + gi
                    for mi in range(4):
                        nc.tensor.matmul(
                            pv[mi],
                            et4m[:, gi, :, mi * P : (mi + 1) * P],
                            xa[:, g],
                            start=(g == 0),
                            stop=(g == NPAIR - 1),
                            perf_mode=DR,
                        )
            # ---- epilogue: out = 1.1*x - 0.1 * pv/S ----
            for mi in range(4):
                jj = h * 4 + mi
                sinv = work.tile([P, 1], f32, tag="sinv")
                nc.vector.reciprocal(sinv, pv[mi][:, D : D + 1])
                res = work.tile([P, D], f32, tag="res")
                nc.vector.tensor_scalar(
                    out=res,
                    in0=pv[mi][:, 0:D],
                    scalar1=sinv,
                    scalar2=-SCALE,
                    op0=MUL,
                    op1=MUL,
                )
                nc.vector.scalar_tensor_tensor(
                    out=res,
                    in0=xo[:, jj],
                    scalar=1.0 + SCALE,
                    in1=res,
                    op0=MUL,
                    op1=ADD,
                )
                nc.sync.dma_start(
                    out=out_d[jj * P : (jj + 1) * P, :], in_=res
                )

    nc.compile()
    return nc


def get_program():
    if "prog" not in _prog_cache:
        _prog_cache["prog"] = _build_program()
    return _prog_cache["prog"]


def make_in_maps(x, edge_index):
    fp8 = ml_dtypes.float8_e4m3
    x = np.asarray(x, dtype=np.float32)
    ei = np.asarray(edge_index)
    r = ei[0].astype(np.int64)
    c = ei[1].astype(np.int64)

    norm = np.sqrt((x * x).sum(axis=1, keepdims=True))
    nx16 = np.asarray((x / np.maximum(norm, 1e-12)) * 16.0, dtype=fp8)
    x8 = np.asarray(x, dtype=fp8)

    in_maps = []
    for k in range(NCORES):
        lo = k * R
        nxr = np.roll(nx16, -lo, axis=0)          # [N, D] fp8
        xar = np.roll(x8, -lo, axis=0)            # [N, D] fp8

        # xt[p, kt, c] = nxr[c, kt*128 + p]
        xt = np.ascontiguousarray(
            nxr.T.reshape(2, P, N).transpose(1, 0, 2)
        )
        # xa[p, g, kt, j] = xar[(2g+kt)*128 + p, j], ones at j=256
        xa = np.empty((P, NPAIR, 2, D + 1), dtype=fp8)
        xa[:, :, :, 0:D] = xar.reshape(NPAIR, 2, P, D).transpose(2, 0, 1, 3)
        xa[:, :, :, D] = fp8(1.0)

        # keep-mask, rolled: mask[c_rolled, m_local] = 0 on edges
        sel = (r >= lo) & (r < lo + R)
        m_local = (r[sel] - lo).astype(np.int64)
        c_rolled = (c[sel] - lo) % N
        mask = np.ones((N, R), dtype=np.uint8)
        mask[c_rolled, m_local] = 0
        # column mapping m = h*512 + b*64 + j  ->  byte[c, h, j] bit b
        mm = mask.reshape(N, 2, 8, 64)
        packed = np.packbits(mm, axis=2, bitorder="little")  # [N, 2, 1, 64]
        packed = packed.reshape(N, 2, 64)
        # bits[p, t, h, w] = u32 view of packed[t*128+p, h, 4w:4w+4]
        bits = (
            packed.reshape(NT, P, 2, 64)
            .transpose(1, 0, 2, 3)
            .copy()
            .view("<u4")
        )
        xo = np.ascontiguousarray(x[lo : lo + R])
        in_maps.append({"xt": xt, "xa": xa, "bits": bits, "xo": xo})
    return in_maps


def run(x, edge_index, trace=False):
    from concourse.bass_utils import run_bass_kernel_spmd

    nc = get_program()
    in_maps = make_in_maps(x, edge_index)
    br = run_bass_kernel_spmd(nc, in_maps, list(range(NCORES)), trace=trace)
    out = np.concatenate(
        [br.results[k]["out"] for k in range(NCORES)], axis=0
    ).astype(np.float32)
    return out, br


def kernel(x, edge_index):
    out, _ = run(x, edge_index, trace=False)
    return out
